# revision 1
# baseline (speedup 1.0000x reference)
"""Trainium2 Bass kernel for MultiHeadAttention (B=4, L=2048, D=512, H=8) + LayerNorm.

Sharding: core = b*2 + half  (b in 0..4, half in 0..2).
Each core computes ALL 8 heads for 1024 query rows of one batch:
  - projections (bf16 matmuls), masked softmax (unnormalized exp, mask as
    multiply-by-keep, normalization folded into output scaling),
  - attention probabilities written as f32 (the large 512 MiB output),
  - attn @ V via xbar-DMA-transposed probabilities, fc, residual + LayerNorm.
No cross-core communication is needed.
"""
import sys

sys.path.insert(0, "/opt/trn_rl_repo")
import numpy as np

import concourse.bass as bass
import concourse.tile as tile
import concourse.mybir as mybir
from concourse import bacc

P = 128
B, L, D, H, DK = 4, 2048, 512, 8, 64
LQ = L // 2          # query rows per core
QC = LQ // P         # 8 query chunks
KC = L // P          # 16 key chunks
DC = D // P          # 4 d_model chunks
LN_EPS = 1e-5
SCALE = 1.0 / 8.0    # 1/sqrt(DK)

f32 = mybir.dt.float32
bf16 = mybir.dt.bfloat16
u8 = mybir.dt.uint8
FT = mybir.ActivationFunctionType
ALU = mybir.AluOpType

# which (h, qc) units run the attn normalize on ACT vs DVE (load balance)
def _norm_on_act(h, qc):
    return (h * QC + qc) % 5 < 3


def _build(nc, tc, io):
    qx, kx, vx, mk, w_d, bfc_d, g_d, be_d, attn_o, y_o = io

    import contextlib

    with contextlib.ExitStack() as ctx:
        consts = ctx.enter_context(tc.tile_pool(name="consts", bufs=1))
        stage = ctx.enter_context(tc.tile_pool(name="stage", bufs=2))
        pbuf = ctx.enter_context(tc.tile_pool(name="pbuf", bufs=3))
        ptb = ctx.enter_context(tc.tile_pool(name="ptb", bufs=3))
        abuf = ctx.enter_context(tc.tile_pool(name="abuf", bufs=2))
        otb = ctx.enter_context(tc.tile_pool(name="otb", bufs=2))
        small = ctx.enter_context(tc.tile_pool(name="small", bufs=8))
        ln = ctx.enter_context(tc.tile_pool(name="ln", bufs=3))
        psA = ctx.enter_context(tc.tile_pool(name="psA", bufs=3, space="PSUM"))
        psO = ctx.enter_context(tc.tile_pool(name="psO", bufs=2, space="PSUM"))

        # ---------------- phase 0: weights / constants ----------------
        wb = {}
        for name in ("w_q", "w_k", "w_v", "w_fc"):
            wtile = consts.tile([P, DC, D], bf16, tag=f"wb_{name}")
            wb[name] = wtile
            wr = w_d[name].ap().rearrange("(c p) n -> c p n", p=P)
            for dmc in range(DC):
                wf = stage.tile([P, D], f32, tag="wf")
                nc.sync.dma_start(out=wf, in_=wr[dmc])
                nc.vector.tensor_copy(out=wtile[:, dmc], in_=wf)

        bfc = consts.tile([P, D], f32, tag="bfc")
        nc.gpsimd.dma_start(
            out=bfc, in_=bass.AP(tensor=bfc_d, offset=0, ap=[[0, P], [1, D]])
        )
        gam = consts.tile([P, D], f32, tag="gam")
        nc.gpsimd.dma_start(
            out=gam, in_=bass.AP(tensor=g_d, offset=0, ap=[[0, P], [1, D]])
        )
        bet = consts.tile([P, D], f32, tag="bet")
        nc.gpsimd.dma_start(
            out=bet, in_=bass.AP(tensor=be_d, offset=0, ap=[[0, P], [1, D]])
        )
        eps_t = consts.tile([P, 1], f32, tag="eps")
        nc.vector.memset(eps_t, LN_EPS)

        # ---------------- phase 0: input transposes + projections ----------------
        qhT = consts.tile([P, DC, LQ], bf16, tag="qhT")
        khT = consts.tile([P, DC, L], bf16, tag="khT")
        vh = consts.tile([P, KC, H, DK + 1], bf16, tag="vh")
        nc.vector.memset(vh[:, :, :, DK : DK + 1], 1.0)

        qxr = qx.rearrange("(c p) d -> c p d", p=P)
        kxr = kx.rearrange("(c p) d -> c p d", p=P)
        vxr = vx.rearrange("(c p) d -> c p d", p=P)

        # q: per tile of 512 rows (4 chunks), transpose then project into qhT
        for t in range(LQ // 512):
            sT = stage.tile([P, DC, 512], bf16, tag="seqT")
            for j in range(4):
                xf = stage.tile([P, D], f32, tag="xf")
                nc.sync.dma_start(out=xf, in_=qxr[4 * t + j])
                xb = stage.tile([P, D], bf16, tag="xb")
                nc.vector.tensor_copy(out=xb, in_=xf)
                nc.scalar.dma_start_transpose(
                    out=sT[:, :, j * P : (j + 1) * P], in_=xb
                )
            for cc in range(DC):
                pt = psA.tile([P, 512], f32, tag="st")
                for dmc in range(DC):
                    nc.tensor.matmul(
                        pt,
                        lhsT=wb["w_q"][:, dmc, cc * P : (cc + 1) * P],
                        rhs=sT[:, dmc],
                        start=(dmc == 0),
                        stop=(dmc == DC - 1),
                    )
                nc.scalar.activation(
                    out=qhT[:, cc, t * 512 : (t + 1) * 512], in_=pt, func=FT.Copy
                )

        # k: same, into khT
        for t in range(L // 512):
            sT = stage.tile([P, DC, 512], bf16, tag="seqT")
            for j in range(4):
                xf = stage.tile([P, D], f32, tag="xf")
                nc.sync.dma_start(out=xf, in_=kxr[4 * t + j])
                xb = stage.tile([P, D], bf16, tag="xb")
                nc.vector.tensor_copy(out=xb, in_=xf)
                nc.scalar.dma_start_transpose(
                    out=sT[:, :, j * P : (j + 1) * P], in_=xb
                )
            for cc in range(DC):
                pt = psA.tile([P, 512], f32, tag="st")
                for dmc in range(DC):
                    nc.tensor.matmul(
                        pt,
                        lhsT=wb["w_k"][:, dmc, cc * P : (cc + 1) * P],
                        rhs=sT[:, dmc],
                        start=(dmc == 0),
                        stop=(dmc == DC - 1),
                    )
                nc.scalar.activation(
                    out=khT[:, cc, t * 512 : (t + 1) * 512], in_=pt, func=FT.Copy
                )

        # v: per 128-row chunk, transpose then project into vh (natural layout)
        for sc in range(KC):
            xf = stage.tile([P, D], f32, tag="xf")
            nc.sync.dma_start(out=xf, in_=vxr[sc])
            xb = stage.tile([P, D], bf16, tag="xb")
            nc.vector.tensor_copy(out=xb, in_=xf)
            vT = stage.tile([P, DC, P], bf16, tag="vT")
            nc.scalar.dma_start_transpose(out=vT, in_=xb)
            pt = psA.tile([P, 512], f32, tag="st")
            for dmc in range(DC):
                nc.tensor.matmul(
                    pt,
                    lhsT=vT[:, dmc],
                    rhs=wb["w_v"][:, dmc],
                    start=(dmc == 0),
                    stop=(dmc == DC - 1),
                )
            nc.scalar.activation(
                out=vh[:, sc, :, 0:DK],
                in_=pt.rearrange("p (h d) -> p h d", h=H),
                func=FT.Copy,
            )

        # keep mask: bf16 keep = 1 - mask
        keep = consts.tile([P, QC, L], bf16, tag="keep")
        mkr = mk.rearrange("(c p) l -> c p l", p=P)
        for c in range(QC):
            mu = stage.tile([P, L], u8, tag="mu8")
            nc.sync.dma_start(out=mu, in_=mkr[c])
            nc.vector.tensor_scalar(
                out=keep[:, c],
                in0=mu,
                scalar1=-1.0,
                scalar2=1.0,
                op0=ALU.mult,
                op1=ALU.add,
            )

        # ---------------- phase 1: attention ----------------
        o_stage = consts.tile([P, QC, H, DK], bf16, tag="o_stage")

        for h in range(H):
            hp = (h % 2) * DK
            hc = h // 2
            for qc in range(QC):
                p_t = pbuf.tile([P, L], bf16, tag="p")
                for half in range(2):
                    st = psA.tile([P, 1024], f32, tag="st")
                    for n in range(2):
                        kt = 2 * half + n
                        nc.tensor.matmul(
                            st[:, n * 512 : (n + 1) * 512],
                            lhsT=qhT[hp : hp + DK, hc, qc * P : (qc + 1) * P],
                            rhs=khT[hp : hp + DK, hc, kt * 512 : (kt + 1) * 512],
                            start=True,
                            stop=True,
                        )
                    nc.scalar.activation(
                        out=p_t[:, half * 1024 : (half + 1) * 1024],
                        in_=st,
                        func=FT.Exp,
                        scale=SCALE,
                    )
                nc.vector.tensor_mul(out=p_t, in0=p_t, in1=keep[:, qc])

                pT = ptb.tile([P, KC, P], bf16, tag="pT")
                nc.scalar.dma_start_transpose(out=pT, in_=p_t)

                o_t = psO.tile([P, DK + 1], f32, tag="o")
                for kc in range(KC):
                    nc.tensor.matmul(
                        o_t,
                        lhsT=pT[:, kc],
                        rhs=vh[:, kc, h],
                        start=(kc == 0),
                        stop=(kc == KC - 1),
                    )
                invd = small.tile([P, 1], f32, tag="invd")
                nc.vector.reciprocal(out=invd, in_=o_t[:, DK : DK + 1])

                at = abuf.tile([P, L], f32, tag="at")
                if _norm_on_act(h, qc):
                    nc.scalar.activation(out=at, in_=p_t, func=FT.Copy, scale=invd)
                else:
                    nc.vector.tensor_scalar_mul(out=at, in0=p_t, scalar1=invd)
                nc.sync.dma_start(
                    out=attn_o[h, qc * P : (qc + 1) * P, :], in_=at
                )
                nc.scalar.activation(
                    out=o_stage[:, qc, h], in_=o_t[:, 0:DK], func=FT.Copy, scale=invd
                )

        # ---------------- phase 2: fc + residual + LayerNorm ----------------
        y_r = y_o.rearrange("(c p) d -> c p d", p=P)
        for qc in range(QC):
            oT = otb.tile([P, DC, P], bf16, tag="oT")
            nc.scalar.dma_start_transpose(
                out=oT, in_=o_stage[:, qc].rearrange("p h d -> p (h d)")
            )
            yp = psO.tile([P, D], f32, tag="o")
            for cc in range(DC):
                nc.tensor.matmul(
                    yp,
                    lhsT=oT[:, cc],
                    rhs=wb["w_fc"][:, cc],
                    start=(cc == 0),
                    stop=(cc == DC - 1),
                )
            res = ln.tile([P, D], f32, tag="res")
            nc.sync.dma_start(out=res, in_=qxr[qc])
            x = ln.tile([P, D], f32, tag="x")
            nc.vector.tensor_add(out=x, in0=yp, in1=res)
            nc.vector.tensor_add(out=x, in0=x, in1=bfc)
            stt = ln.tile([P, nc.vector.BN_STATS_DIM], f32, tag="stt")
            nc.vector.bn_stats(out=stt, in_=x)
            mv = ln.tile([P, nc.vector.BN_AGGR_DIM], f32, tag="mv")
            nc.vector.bn_aggr(out=mv, in_=stt)
            sd = small.tile([P, 1], f32, tag="sd")
            nc.scalar.activation(
                out=sd, in_=mv[:, 1:2], func=FT.Sqrt, bias=eps_t, scale=1.0
            )
            rstd = small.tile([P, 1], f32, tag="rstd")
            nc.vector.reciprocal(out=rstd, in_=sd)
            t1 = ln.tile([P, D], f32, tag="t1")
            nc.vector.tensor_scalar(
                out=t1,
                in0=x,
                scalar1=mv[:, 0:1],
                scalar2=rstd,
                op0=ALU.subtract,
                op1=ALU.mult,
            )
            nc.vector.tensor_mul(out=t1, in0=t1, in1=gam)
            nc.vector.tensor_add(out=t1, in0=t1, in1=bet)
            nc.sync.dma_start(out=y_r[qc], in_=t1)


def build_module():
    nc = bacc.Bacc("TRN2", debug=False)
    qx = nc.dram_tensor("qx", [LQ, D], f32, kind="ExternalInput").ap()
    kx = nc.dram_tensor("kx", [L, D], f32, kind="ExternalInput").ap()
    vx = nc.dram_tensor("vx", [L, D], f32, kind="ExternalInput").ap()
    mk = nc.dram_tensor("mask", [LQ, L], u8, kind="ExternalInput").ap()
    w_d = {
        name: nc.dram_tensor(name, [D, D], f32, kind="ExternalInput")
        for name in ("w_q", "w_k", "w_v", "w_fc")
    }
    bfc_d = nc.dram_tensor("b_fc", [D], f32, kind="ExternalInput")
    g_d = nc.dram_tensor("ln_g", [D], f32, kind="ExternalInput")
    be_d = nc.dram_tensor("ln_b", [D], f32, kind="ExternalInput")
    attn_o = nc.dram_tensor("attn_out", [H, LQ, L], f32, kind="ExternalOutput").ap()
    y_o = nc.dram_tensor("y_out", [LQ, D], f32, kind="ExternalOutput").ap()

    with tile.TileContext(nc) as tc:
        _build(nc, tc, (qx, kx, vx, mk, w_d, bfc_d, g_d, be_d, attn_o, y_o))
    nc.compile()
    return nc


_NC = None


def _get_nc():
    global _NC
    if _NC is None:
        _NC = build_module()
    return _NC


def make_in_maps(q, k, v, mask, w_q, w_k, w_v, w_fc, b_fc, ln_gamma, ln_beta):
    q = np.asarray(q, np.float32)
    k = np.asarray(k, np.float32)
    v = np.asarray(v, np.float32)
    mask_u8 = np.asarray(mask).astype(np.uint8)
    common = {
        "w_q": np.asarray(w_q, np.float32),
        "w_k": np.asarray(w_k, np.float32),
        "w_v": np.asarray(w_v, np.float32),
        "w_fc": np.asarray(w_fc, np.float32),
        "b_fc": np.asarray(b_fc, np.float32),
        "ln_g": np.asarray(ln_gamma, np.float32),
        "ln_b": np.asarray(ln_beta, np.float32),
    }
    in_maps = []
    for core in range(8):
        b, hf = core // 2, core % 2
        sl = slice(hf * LQ, (hf + 1) * LQ)
        in_maps.append(
            dict(
                common,
                qx=np.ascontiguousarray(q[b, sl]),
                kx=np.ascontiguousarray(k[b]),
                vx=np.ascontiguousarray(v[b]),
                mask=np.ascontiguousarray(mask_u8[b, sl]),
            )
        )
    return in_maps


def assemble(results):
    y = np.empty((B, L, D), np.float32)
    attn = np.empty((H * B, L, L), np.float32)
    for core in range(8):
        b, hf = core // 2, core % 2
        sl = slice(hf * LQ, (hf + 1) * LQ)
        r = results[core]
        y[b, sl] = r["y_out"]
        for h in range(H):
            attn[h * B + b, sl] = r["attn_out"][h]
    return y, attn


def run(trace=False, **inputs):
    from concourse.bass_utils import run_bass_kernel_spmd

    nc = _get_nc()
    in_maps = make_in_maps(**inputs)
    res = run_bass_kernel_spmd(nc, in_maps, core_ids=list(range(8)), trace=trace)
    return assemble(res.results), res


def kernel(**inputs):
    (y, attn), _ = run(trace=False, **inputs)
    return y, attn


# revision 9
# speedup vs baseline: 2.5650x; 2.5650x over previous
"""Trainium2 Bass kernel for MultiHeadAttention (B=4, L=2048, D=512, H=8) + LayerNorm.

Sharding: core = b*2 + half  (b in 0..4, half in 0..2).
Each core computes ALL 8 heads for 1024 query rows of one batch:
  - projections (bf16 matmuls), masked softmax (unnormalized exp, mask as
    multiply-by-keep, normalization folded into output scaling),
  - attention probabilities written as f32 (the large 512 MiB output),
  - attn @ V via xbar-DMA-transposed probabilities, fc, residual + LayerNorm.
No cross-core communication is needed.
"""
import sys

sys.path.insert(0, "/opt/trn_rl_repo")
import numpy as np

import concourse.bass as bass
import concourse.tile as tile
import concourse.mybir as mybir
from concourse import bacc

P = 128
B, L, D, H, DK = 4, 2048, 512, 8, 64
LQ = L // 2          # query rows per core
QC = LQ // P         # 8 query chunks
KC = L // P          # 16 key chunks
DC = D // P          # 4 d_model chunks
LN_EPS = 1e-5
SCALE = 1.0 / 8.0    # 1/sqrt(DK)

f32 = mybir.dt.float32
bf16 = mybir.dt.bfloat16
u8 = mybir.dt.uint8
FT = mybir.ActivationFunctionType
ALU = mybir.AluOpType

# which (h, qc) units run the attn normalize on ACT vs DVE (load balance)
def _norm_on_act(h, qc):
    return (h * QC + qc) % 4 == 0


def _build(nc, tc, io):
    qx, kx, vx, mk, w_d, bfc_d, g_d, be_d, attn_o, y_o = io

    import contextlib
    import os as _os

    big = nc.gpsimd if _os.environ.get("KDMA", "sw") == "sw" else nc.sync

    with contextlib.ExitStack() as ctx:
        consts = ctx.enter_context(tc.tile_pool(name="consts", bufs=1))
        stage = ctx.enter_context(tc.tile_pool(name="stage", bufs=2))
        pbuf = ctx.enter_context(tc.tile_pool(name="pbuf", bufs=3))
        ptb = ctx.enter_context(tc.tile_pool(name="ptb", bufs=3))
        abuf = ctx.enter_context(tc.tile_pool(name="abuf", bufs=2))
        otb = ctx.enter_context(tc.tile_pool(name="otb", bufs=2))
        small = ctx.enter_context(tc.tile_pool(name="small", bufs=8))
        ln = ctx.enter_context(tc.tile_pool(name="ln", bufs=2))
        psA = ctx.enter_context(tc.tile_pool(name="psA", bufs=3, space="PSUM"))
        psO = ctx.enter_context(tc.tile_pool(name="psO", bufs=2, space="PSUM"))

        # ---------------- phase 0: weights / constants ----------------
        wb = {}
        for name in ("w_q", "w_k", "w_v", "w_fc"):
            wtile = consts.tile([P, DC, D], bf16, tag=f"wb_{name}")
            wb[name] = wtile
            wr = w_d[name].ap().rearrange("(c p) n -> c p n", p=P)
            for dmc in range(DC):
                wf = stage.tile([P, D], f32, tag="wf")
                nc.sync.dma_start(out=wf, in_=wr[dmc])
                nc.vector.tensor_copy(out=wtile[:, dmc], in_=wf)

        bfc = consts.tile([P, D], f32, tag="bfc")
        nc.gpsimd.dma_start(
            out=bfc, in_=bass.AP(tensor=bfc_d, offset=0, ap=[[0, P], [1, D]])
        )
        gam = consts.tile([P, D], f32, tag="gam")
        nc.gpsimd.dma_start(
            out=gam, in_=bass.AP(tensor=g_d, offset=0, ap=[[0, P], [1, D]])
        )
        bet = consts.tile([P, D], f32, tag="bet")
        nc.gpsimd.dma_start(
            out=bet, in_=bass.AP(tensor=be_d, offset=0, ap=[[0, P], [1, D]])
        )
        eps_t = consts.tile([P, 1], f32, tag="eps")
        nc.vector.memset(eps_t, LN_EPS)

        # ---------------- phase 0: input transposes + projections ----------------
        qhT = consts.tile([P, DC, LQ], bf16, tag="qhT")
        khT = consts.tile([P, DC, L], bf16, tag="khT")
        vh = consts.tile([P, KC, H, DK + 1], bf16, tag="vh")
        nc.vector.memset(vh[:, :, :, DK : DK + 1], 1.0)

        qxr = qx.rearrange("(c p) d -> c p d", p=P)
        kxr = kx.rearrange("(c p) d -> c p d", p=P)
        vxr = vx.rearrange("(c p) d -> c p d", p=P)

        # q: per tile of 512 rows (4 chunks), transpose then project into qhT
        for t in range(LQ // 512):
            sT = stage.tile([P, DC, 512], bf16, tag="seqT")
            for jp in range(2):
                xf = stage.tile([P, 2, D], f32, tag="xf")
                big.dma_start(out=xf, in_=qxr[4 * t + 2 * jp : 4 * t + 2 * jp + 2].rearrange("c p d -> p c d"))
                xb = stage.tile([P, 2, D], bf16, tag="xb")
                nc.vector.tensor_copy(out=xb, in_=xf)
                for jj in range(2):
                    j = 2 * jp + jj
                    eng = nc.scalar if j == 0 else nc.sync
                    eng.dma_start_transpose(
                        out=sT[:, :, j * P : (j + 1) * P], in_=xb[:, jj]
                    )
            for cc in range(DC):
                pt = psA.tile([P, 512], f32, tag="st")
                for dmc in range(DC):
                    nc.tensor.matmul(
                        pt,
                        lhsT=wb["w_q"][:, dmc, cc * P : (cc + 1) * P],
                        rhs=sT[:, dmc],
                        start=(dmc == 0),
                        stop=(dmc == DC - 1),
                    )
                nc.vector.tensor_copy(
                    out=qhT[:, cc, t * 512 : (t + 1) * 512], in_=pt
                )

        # k: same, into khT
        for t in range(L // 512):
            sT = stage.tile([P, DC, 512], bf16, tag="seqT")
            for jp in range(2):
                xf = stage.tile([P, 2, D], f32, tag="xf")
                big.dma_start(out=xf, in_=kxr[4 * t + 2 * jp : 4 * t + 2 * jp + 2].rearrange("c p d -> p c d"))
                xb = stage.tile([P, 2, D], bf16, tag="xb")
                nc.vector.tensor_copy(out=xb, in_=xf)
                for jj in range(2):
                    j = 2 * jp + jj
                    eng = nc.scalar if j == 0 else nc.sync
                    eng.dma_start_transpose(
                        out=sT[:, :, j * P : (j + 1) * P], in_=xb[:, jj]
                    )
            for cc in range(DC):
                pt = psA.tile([P, 512], f32, tag="st")
                for dmc in range(DC):
                    nc.tensor.matmul(
                        pt,
                        lhsT=wb["w_k"][:, dmc, cc * P : (cc + 1) * P],
                        rhs=sT[:, dmc],
                        start=(dmc == 0),
                        stop=(dmc == DC - 1),
                    )
                nc.vector.tensor_copy(
                    out=khT[:, cc, t * 512 : (t + 1) * 512], in_=pt
                )

        # v: per 128-row chunk, transpose then project into vh (natural layout)
        for sc in range(KC):
            xf = stage.tile([P, D], f32, tag="xf")
            big.dma_start(out=xf, in_=vxr[sc])
            xb = stage.tile([P, D], bf16, tag="xb")
            nc.vector.tensor_copy(out=xb, in_=xf)
            vT = stage.tile([P, DC, P], bf16, tag="vT")
            (nc.scalar if sc % 2 == 0 else nc.sync).dma_start_transpose(out=vT, in_=xb)
            pt = psA.tile([P, 512], f32, tag="st")
            for dmc in range(DC):
                nc.tensor.matmul(
                    pt,
                    lhsT=vT[:, dmc],
                    rhs=wb["w_v"][:, dmc],
                    start=(dmc == 0),
                    stop=(dmc == DC - 1),
                )
            nc.scalar.activation(
                out=vh[:, sc, :, 0:DK],
                in_=pt.rearrange("p (h d) -> p h d", h=H),
                func=FT.Copy,
            )

        # keep mask: bf16 keep = 1 - mask
        keep = consts.tile([P, QC, L], bf16, tag="keep")
        mkr = mk.rearrange("(c p) l -> c p l", p=P)
        for c in range(QC):
            mu = stage.tile([P, L], u8, tag="mu8")
            nc.sync.dma_start(out=mu, in_=mkr[c])
            nc.vector.tensor_scalar(
                out=keep[:, c],
                in0=mu,
                scalar1=-1.0,
                scalar2=1.0,
                op0=ALU.mult,
                op1=ALU.add,
            )

        # ---------------- phase 1: attention ----------------
        o_stage = consts.tile([P, QC, H, DK], bf16, tag="o_stage")

        for h in range(H):
            hp = (h % 2) * DK
            hc = h // 2
            at_pair = None
            for qc in range(QC):
                p_t = pbuf.tile([P, L], bf16, tag="p")
                for half in range(2):
                    st = psA.tile([P, 1024], f32, tag="st")
                    for n in range(2):
                        kt = 2 * half + n
                        nc.tensor.matmul(
                            st[:, n * 512 : (n + 1) * 512],
                            lhsT=qhT[hp : hp + DK, hc, qc * P : (qc + 1) * P],
                            rhs=khT[hp : hp + DK, hc, kt * 512 : (kt + 1) * 512],
                            start=True,
                            stop=True,
                        )
                    nc.scalar.activation(
                        out=p_t[:, half * 1024 : (half + 1) * 1024],
                        in_=st,
                        func=FT.Exp,
                        scale=SCALE,
                    )
                nc.vector.tensor_mul(out=p_t, in0=p_t, in1=keep[:, qc])

                import os as _os

                _var = _os.environ.get("KVAR", "").split(",")
                if "notrans" in _var:
                    # timing experiment: skip transpose, feed p_t directly
                    # (wrong av values, same matmul cost)
                    pT = p_t.rearrange("p (c q) -> p c q", c=KC)
                else:
                    pT = ptb.tile([P, KC, P], bf16, tag="pT")
                    eng = nc.scalar if (h * QC + qc) % 3 == 0 else nc.sync
                    eng.dma_start_transpose(out=pT, in_=p_t)

                o_t = psO.tile([P, DK + 1], f32, tag="o")
                for kc in range(KC):
                    nc.tensor.matmul(
                        o_t,
                        lhsT=pT[:, kc],
                        rhs=vh[:, kc, h],
                        start=(kc == 0),
                        stop=(kc == KC - 1),
                    )
                invd = small.tile([P, 1], f32, tag="invd")
                nc.vector.reciprocal(out=invd, in_=o_t[:, DK : DK + 1])

                if qc % 2 == 0:
                    at_pair = abuf.tile([P, 2, L], f32, tag="at")
                at = at_pair[:, qc % 2]
                if _norm_on_act(h, qc):
                    nc.scalar.activation(out=at, in_=p_t, func=FT.Copy, scale=invd)
                else:
                    nc.vector.tensor_scalar_mul(out=at, in0=p_t, scalar1=invd)
                if "nodma" not in _var and qc % 2 == 1:
                    big.dma_start(
                        out=attn_o[h, (qc - 1) * P : (qc + 1) * P, :].rearrange(
                            "(c p) l -> p c l", p=P
                        ),
                        in_=at_pair,
                    )
                nc.scalar.activation(
                    out=o_stage[:, qc, h], in_=o_t[:, 0:DK], func=FT.Copy, scale=invd
                )

        # ---------------- phase 2: fc + residual + LayerNorm ----------------
        y_r = y_o.rearrange("(c p) d -> c p d", p=P)
        for qc in range(QC):
            oT = otb.tile([P, DC, P], bf16, tag="oT")
            (nc.scalar if qc % 2 == 0 else nc.sync).dma_start_transpose(
                out=oT, in_=o_stage[:, qc].rearrange("p h d -> p (h d)")
            )
            yp = psO.tile([P, D], f32, tag="o")
            for cc in range(DC):
                nc.tensor.matmul(
                    yp,
                    lhsT=oT[:, cc],
                    rhs=wb["w_fc"][:, cc],
                    start=(cc == 0),
                    stop=(cc == DC - 1),
                )
            res = ln.tile([P, D], f32, tag="res")
            nc.sync.dma_start(out=res, in_=qxr[qc])
            x = ln.tile([P, D], f32, tag="x")
            nc.vector.tensor_add(out=x, in0=yp, in1=res)
            nc.vector.tensor_add(out=x, in0=x, in1=bfc)
            stt = ln.tile([P, nc.vector.BN_STATS_DIM], f32, tag="stt")
            nc.vector.bn_stats(out=stt, in_=x)
            mv = ln.tile([P, nc.vector.BN_AGGR_DIM], f32, tag="mv")
            nc.vector.bn_aggr(out=mv, in_=stt)
            sd = small.tile([P, 1], f32, tag="sd")
            nc.scalar.activation(
                out=sd, in_=mv[:, 1:2], func=FT.Sqrt, bias=eps_t, scale=1.0
            )
            rstd = small.tile([P, 1], f32, tag="rstd")
            nc.vector.reciprocal(out=rstd, in_=sd)
            t1 = ln.tile([P, D], f32, tag="t1")
            nc.vector.tensor_scalar(
                out=t1,
                in0=x,
                scalar1=mv[:, 0:1],
                scalar2=rstd,
                op0=ALU.subtract,
                op1=ALU.mult,
            )
            nc.vector.tensor_mul(out=t1, in0=t1, in1=gam)
            nc.vector.tensor_add(out=t1, in0=t1, in1=bet)
            nc.sync.dma_start(out=y_r[qc], in_=t1)


def build_module():
    nc = bacc.Bacc("TRN2", debug=False)
    qx = nc.dram_tensor("qx", [LQ, D], f32, kind="ExternalInput").ap()
    kx = nc.dram_tensor("kx", [L, D], f32, kind="ExternalInput").ap()
    vx = nc.dram_tensor("vx", [L, D], f32, kind="ExternalInput").ap()
    mk = nc.dram_tensor("mask", [LQ, L], u8, kind="ExternalInput").ap()
    w_d = {
        name: nc.dram_tensor(name, [D, D], f32, kind="ExternalInput")
        for name in ("w_q", "w_k", "w_v", "w_fc")
    }
    bfc_d = nc.dram_tensor("b_fc", [D], f32, kind="ExternalInput")
    g_d = nc.dram_tensor("ln_g", [D], f32, kind="ExternalInput")
    be_d = nc.dram_tensor("ln_b", [D], f32, kind="ExternalInput")
    attn_o = nc.dram_tensor("attn_out", [H, LQ, L], f32, kind="ExternalOutput").ap()
    y_o = nc.dram_tensor("y_out", [LQ, D], f32, kind="ExternalOutput").ap()

    with tile.TileContext(nc) as tc:
        _build(nc, tc, (qx, kx, vx, mk, w_d, bfc_d, g_d, be_d, attn_o, y_o))
    nc.compile()
    return nc


_NC = None


def _get_nc():
    global _NC
    if _NC is None:
        _NC = build_module()
    return _NC


def make_in_maps(q, k, v, mask, w_q, w_k, w_v, w_fc, b_fc, ln_gamma, ln_beta):
    q = np.asarray(q, np.float32)
    k = np.asarray(k, np.float32)
    v = np.asarray(v, np.float32)
    mask_u8 = np.asarray(mask).astype(np.uint8)
    common = {
        "w_q": np.asarray(w_q, np.float32),
        "w_k": np.asarray(w_k, np.float32),
        "w_v": np.asarray(w_v, np.float32),
        "w_fc": np.asarray(w_fc, np.float32),
        "b_fc": np.asarray(b_fc, np.float32),
        "ln_g": np.asarray(ln_gamma, np.float32),
        "ln_b": np.asarray(ln_beta, np.float32),
    }
    in_maps = []
    for core in range(8):
        b, hf = core // 2, core % 2
        sl = slice(hf * LQ, (hf + 1) * LQ)
        in_maps.append(
            dict(
                common,
                qx=np.ascontiguousarray(q[b, sl]),
                kx=np.ascontiguousarray(k[b]),
                vx=np.ascontiguousarray(v[b]),
                mask=np.ascontiguousarray(mask_u8[b, sl]),
            )
        )
    return in_maps


def assemble(results):
    y = np.empty((B, L, D), np.float32)
    attn = np.empty((H * B, L, L), np.float32)
    for core in range(8):
        b, hf = core // 2, core % 2
        sl = slice(hf * LQ, (hf + 1) * LQ)
        r = results[core]
        y[b, sl] = r["y_out"]
        for h in range(H):
            attn[h * B + b, sl] = r["attn_out"][h]
    return y, attn


def run(trace=False, **inputs):
    from concourse.bass_utils import run_bass_kernel_spmd

    nc = _get_nc()
    in_maps = make_in_maps(**inputs)
    res = run_bass_kernel_spmd(nc, in_maps, core_ids=list(range(8)), trace=trace)
    return assemble(res.results), res


def kernel(**inputs):
    (y, attn), _ = run(trace=False, **inputs)
    return y, attn
